# revision 4
# baseline (speedup 1.0000x reference)
"""Causal self-attention (S=2048, D=1024, 16 heads x 64) on 8 Trainium2 cores.

Tensor-parallel sharding: 2 heads per core. Each core computes
  qkv_local = x @ Wqkv[:, local]      (local q/k/v columns, q pre-scaled 1/8)
  attn_h    = softmax(mask(q_h k_h^T)) v_h          for its 2 heads
  partial   = concat(attn) @ Wout[local_rows, :]    (128 rows of Wout)
and the host sums the 8 partials (+bias).

On-chip layout: q^T/k^T are [128, S] with head-dim on partitions (h0 rows
0-63, h1 rows 64-127) so the two heads' logit matmuls (K=64 contractions)
run concurrently in different PE row-groups. Logits are computed
transposed ([key, query]) so that exp(logits) blocks feed the
probs@v matmul directly as the moving operand; v carries an appended
ones-column so the same accumulation also produces the softmax row-sums.
No max-subtraction is needed: logits are ~N(0,1) after the 1/8 scale, so
exp() is well within fp32 range, and masked entries are zeroed after exp.
"""

import numpy as np

import concourse.bass as bass
import concourse.mybir as mybir
import concourse.tile as tile
from concourse import bacc
from concourse.bass_utils import run_bass_kernel_spmd

S = 2048
D = 1024
DH = 64
N_CORES = 8

P = 128
NB512 = S // 512  # 512-wide query chunks
NB128 = S // 128  # 128-wide chunks
KO = D // P  # contraction chunks for the projections

F32 = mybir.dt.float32

_compiled = {}


def _emit(nc, tc, mm_dt, xt, w, wout, maskt, out):
    f32 = F32
    with (
        tc.tile_pool(name="const", bufs=1) as const,
        tc.tile_pool(name="epool", bufs=6) as epool,
        tc.tile_pool(name="rpool", bufs=4) as rpool,
        tc.tile_pool(name="opool", bufs=3) as opool,
        tc.tile_pool(name="psmm", bufs=4, space="PSUM") as psmm,
        tc.tile_pool(name="psacc", bufs=2, space="PSUM") as psacc,
        tc.tile_pool(name="psbc", bufs=1, space="PSUM") as psbc,
    ):
        sb_xT = const.tile([P, KO, S], mm_dt, name="sb_xT")
        sb_w = const.tile([P, KO, 384], mm_dt, name="sb_w")
        sb_wout = const.tile([P, D], mm_dt, name="sb_wout")
        sb_mask = const.tile([P, 4, 512], mm_dt, name="sb_mask")
        sb_qT = const.tile([P, S], mm_dt, name="sb_qT")
        sb_kT = const.tile([P, S], mm_dt, name="sb_kT")
        sb_v = const.tile([P, NB128, 130], mm_dt, name="sb_v")
        sb_attnT = const.tile([P, S], mm_dt, name="sb_attnT")
        sb_ones = const.tile([1, DH], f32, name="sb_ones")

        # loads (per-chunk so compute can start early)
        for o in range(KO):
            nc.sync.dma_start(sb_xT[:, o, :], xt[o * P : (o + 1) * P, :])
            nc.sync.dma_start(sb_w[:, o, :], w[o * P : (o + 1) * P, :])
        nc.sync.dma_start(sb_wout[:], wout[:])
        nc.sync.dma_start(sb_mask[:], maskt[:])
        nc.gpsimd.memset(sb_ones[:], 1.0)
        nc.gpsimd.memset(sb_v[:, :, DH].bitcast(F32), 1.0)
        nc.gpsimd.memset(sb_v[:, :, 129].bitcast(F32), 1.0)

        # q^T / k^T : [c, s] = sum_D W[D, c] * xT[D, s]
        for cc, dest in ((0, sb_qT), (1, sb_kT)):
            for si in range(NB512):
                ps = psmm.tile([P, 512], f32, name="ps_qk", tag="mm")
                for o in range(KO):
                    nc.tensor.matmul(
                        ps[:],
                        sb_w[:, o, cc * P : (cc + 1) * P],
                        sb_xT[:, o, si * 512 : (si + 1) * 512],
                        start=(o == 0),
                        stop=(o == KO - 1),
                    )
                nc.vector.tensor_copy(dest[:, si * 512 : (si + 1) * 512], ps[:])

        # v natural: [s, d] = sum_D xT[D, s] * Wv[D, d]  (both heads: 128 cols)
        for sc in range(NB128):
            psv = psmm.tile([P, 512], f32, name="ps_v", tag="mm")
            for o in range(KO):
                nc.tensor.matmul(
                    psv[:, :P],
                    sb_xT[:, o, sc * P : (sc + 1) * P],
                    sb_w[:, o, 256:384],
                    start=(o == 0),
                    stop=(o == KO - 1),
                )
            nc.vector.tensor_copy(sb_v[:, sc, 0:DH], psv[:, 0:DH])
            nc.vector.tensor_copy(sb_v[:, sc, DH + 1 : 129], psv[:, DH:P])

        # attention, causal-skipped, logits kept transposed [j, i]
        for ic in range(NB512):
            for h in (0, 1):
                po = h * DH
                acc = psacc.tile([DH + 1, 512], f32, name="ps_acc", tag="acc")
                njc = 4 * (ic + 1)
                for jc in range(njc):
                    pl = psmm.tile([P, 512], f32, name="ps_l", tag="mm")
                    nc.tensor.matmul(
                        pl[:],
                        sb_kT[po : po + DH, jc * P : (jc + 1) * P],
                        sb_qT[po : po + DH, ic * 512 : (ic + 1) * 512],
                        start=True,
                        stop=True,
                    )
                    e = epool.tile([P, 512], mm_dt, name="e_t", tag="e")
                    nc.scalar.activation(
                        e[:], pl[:], mybir.ActivationFunctionType.Exp
                    )
                    r = jc - 4 * ic
                    if r >= 0:
                        nc.vector.tensor_mul(e[:], e[:], sb_mask[:, r, :])
                    nc.tensor.matmul(
                        acc[:],
                        sb_v[:, jc, h * 65 : (h + 1) * 65],
                        e[:],
                        start=(jc == 0),
                        stop=(jc == njc - 1),
                    )
                # normalize: attnT = acc[0:64] * (1/rowsum) broadcast over d
                rc = rpool.tile([1, 512], f32, name="rc", tag="rc")
                nc.vector.reciprocal(rc[:], acc[DH : DH + 1, :])
                bc = psbc.tile([DH, 512], f32, name="ps_bc", tag="bc")
                nc.tensor.matmul(bc[:], sb_ones[:], rc[:], start=True, stop=True)
                dst = sb_attnT[po : po + DH, ic * 512 : (ic + 1) * 512]
                nc.scalar.copy(dst, acc[0:DH, :])
                nc.vector.tensor_mul(dst, dst, bc[:])

        # output projection: out[s, e] = sum_d attnT[d, s] * Wout[d, e]
        for sc in range(NB128):
            for ec in range(D // 512):
                pp = psmm.tile([P, 512], f32, name="ps_p", tag="mm")
                nc.tensor.matmul(
                    pp[:],
                    sb_attnT[:, sc * P : (sc + 1) * P],
                    sb_wout[:, ec * 512 : (ec + 1) * 512],
                    start=True,
                    stop=True,
                )
                ot = opool.tile([P, 512], F32, name="ot", tag="ot")
                nc.vector.tensor_copy(ot[:], pp[:])
                nc.sync.dma_start(
                    out[sc * P : (sc + 1) * P, ec * 512 : (ec + 1) * 512], ot[:]
                )


def build(mm_dt=F32):
    key = str(mm_dt)
    if key in _compiled:
        return _compiled[key]
    nc = bacc.Bacc("TRN2", target_bir_lowering=False, debug=False, num_devices=N_CORES)
    xt = nc.dram_tensor("xt", [D, S], mm_dt, kind="ExternalInput").ap()
    w = nc.dram_tensor("w", [D, 384], mm_dt, kind="ExternalInput").ap()
    wout = nc.dram_tensor("wout", [P, D], mm_dt, kind="ExternalInput").ap()
    maskt = nc.dram_tensor("maskt", [P, 4, 512], mm_dt, kind="ExternalInput").ap()
    out = nc.dram_tensor("out", [S, D], F32, kind="ExternalOutput").ap()
    with tile.TileContext(nc) as tc:
        _emit(nc, tc, mm_dt, xt, w, wout, maskt, out)
    nc.compile()
    _compiled[key] = nc
    return nc


def make_inputs(x, Wqkv, Wout):
    """Host-side shard/layout prep -> per-core input maps."""
    x = np.ascontiguousarray(np.asarray(x, np.float32))
    Wqkv = np.asarray(Wqkv, np.float32)
    Wout = np.asarray(Wout, np.float32)
    xT = np.ascontiguousarray(x.T)  # [D, S]
    j = np.arange(512, dtype=np.int64)
    m512 = (j[:, None] <= j[None, :]).astype(np.float32)  # [J, i]: J <= i
    mask = np.ascontiguousarray(
        m512.reshape(4, 128, 512).transpose(1, 0, 2)
    )  # [p, r, i] = (128r + p <= i)
    in_maps = []
    for c in range(N_CORES):
        wq = Wqkv[:, 128 * c : 128 * (c + 1)] * (1.0 / np.sqrt(DH))
        wk = Wqkv[:, D + 128 * c : D + 128 * (c + 1)]
        wv = Wqkv[:, 2 * D + 128 * c : 2 * D + 128 * (c + 1)]
        w_loc = np.ascontiguousarray(np.concatenate([wq, wk, wv], axis=1))
        wout_loc = np.ascontiguousarray(Wout[128 * c : 128 * (c + 1), :])
        in_maps.append(
            {
                "xt": xT,
                "w": w_loc.astype(np.float32),
                "wout": wout_loc,
                "maskt": mask,
            }
        )
    return in_maps


def kernel(x, Wqkv, Wout, bias, mm_dt=F32, **run_kwargs):
    nc = build(mm_dt)
    in_maps = make_inputs(x, Wqkv, Wout)
    res = run_bass_kernel_spmd(nc, in_maps, core_ids=list(range(N_CORES)), **run_kwargs)
    acc = np.zeros((S, D), np.float64)
    for c in range(N_CORES):
        acc += res.results[c]["out"].astype(np.float64)
    acc += np.asarray(bias, np.float64)[None, :]
    return acc.astype(np.float32)


# revision 16
# speedup vs baseline: 1.0229x; 1.0229x over previous
"""Causal self-attention (S=2048, D=1024, 16 heads x 64) on 8 Trainium2 cores.

Tensor-parallel sharding: 2 heads per core. Each core computes
  qkv_local = x @ Wqkv[:, local]      (local q/k/v columns, q pre-scaled 1/8)
  attn_h    = softmax(mask(q_h k_h^T)) v_h          for its 2 heads
  partial   = concat(attn) @ Wout[local_rows, :]    (128 rows of Wout)
and the host sums the 8 partials (+bias).

On-chip layout: q^T/k^T are [128, S] with head-dim on partitions (h0 rows
0-63, h1 rows 64-127) so the two heads' logit matmuls (K=64 contractions)
run concurrently in different PE row-groups. Logits are computed
transposed ([key, query]) so that exp(logits) blocks feed the
probs@v matmul directly as the moving operand; v carries an appended
ones-column so the same accumulation also produces the softmax row-sums.
No max-subtraction is needed: logits are ~N(0,1) after the 1/8 scale, so
exp() is well within fp32 range, and masked entries are zeroed after exp.
"""

import numpy as np

import concourse.bass as bass
import concourse.mybir as mybir
import concourse.tile as tile
from concourse import bacc
from concourse.bass_utils import run_bass_kernel_spmd

S = 2048
D = 1024
DH = 64
N_CORES = 8

P = 128
NB512 = S // 512  # 512-wide query chunks
NB128 = S // 128  # 128-wide chunks
KO = D // P  # contraction chunks for the projections

F32 = mybir.dt.float32

_compiled = {}


def _emit(nc, tc, mm_dt, xt, w, wout, maskt, out):
    f32 = F32
    with (
        tc.tile_pool(name="const", bufs=1) as const,
        tc.tile_pool(name="epool", bufs=18) as epool,
        tc.tile_pool(name="opool", bufs=4) as opool,
        tc.tile_pool(name="rcpool", bufs=1) as rcpool,
        tc.tile_pool(name="psmm", bufs=4, space="PSUM") as psmm,
        tc.tile_pool(name="psacc", bufs=2, space="PSUM") as psacc,
        tc.tile_pool(name="psbc", bufs=1, space="PSUM") as psbc,
    ):
        sb_xT = const.tile([P, KO, S], mm_dt, name="sb_xT")
        sb_w = const.tile([P, KO, 384], mm_dt, name="sb_w")
        sb_wout = const.tile([P, D], mm_dt, name="sb_wout")
        sb_mask = const.tile([P, 4, 512], mm_dt, name="sb_mask")
        sb_qT = const.tile([P, S], mm_dt, name="sb_qT")
        sb_kT = const.tile([P, S], mm_dt, name="sb_kT")
        sb_v = const.tile([P, NB128, 130], mm_dt, name="sb_v")
        sb_attnT = const.tile([P, S], mm_dt, name="sb_attnT")
        sb_ones = const.tile([1, DH], f32, name="sb_ones")

        # loads (per-chunk so compute can start early)
        for o in range(KO):
            nc.sync.dma_start(sb_xT[:, o, :], xt[o * P : (o + 1) * P, :])
            nc.sync.dma_start(sb_w[:, o, :], w[o * P : (o + 1) * P, :])
        nc.sync.dma_start(sb_wout[:], wout[:])
        nc.sync.dma_start(sb_mask[:], maskt[:])
        nc.gpsimd.memset(sb_ones[:], 1.0)
        nc.gpsimd.memset(sb_v[:, :, DH].bitcast(F32), 1.0)
        nc.gpsimd.memset(sb_v[:, :, 129].bitcast(F32), 1.0)

        # q^T / k^T : [c, s] = sum_D W[D, c] * xT[D, s]
        for cc, dest in ((0, sb_qT), (1, sb_kT)):
            for si in range(NB512):
                ps = psmm.tile([P, 512], f32, name="ps_qk", tag="mm")
                for o in range(KO):
                    nc.tensor.matmul(
                        ps[:],
                        sb_w[:, o, cc * P : (cc + 1) * P],
                        sb_xT[:, o, si * 512 : (si + 1) * 512],
                        start=(o == 0),
                        stop=(o == KO - 1),
                    )
                nc.vector.tensor_copy(dest[:, si * 512 : (si + 1) * 512], ps[:])

        # v natural: [s, d] = sum_D xT[D, s] * Wv[D, d]  (both heads: 128 cols)
        for sc in range(NB128):
            psv = psmm.tile([P, 512], f32, name="ps_v", tag="mm")
            for o in range(KO):
                nc.tensor.matmul(
                    psv[:, :P],
                    sb_xT[:, o, sc * P : (sc + 1) * P],
                    sb_w[:, o, 256:384],
                    start=(o == 0),
                    stop=(o == KO - 1),
                )
            nc.vector.tensor_copy(sb_v[:, sc, 0:DH], psv[:, 0:DH])
            nc.vector.tensor_copy(sb_v[:, sc, DH + 1 : 129], psv[:, DH:P])

        # attention, causal-skipped, logits kept transposed [j, i]
        rcs = {}
        for ic in range(NB512):
            for h in (0, 1):
                po = h * DH
                njc = 4 * (ic + 1)
                es = []
                for jc in range(njc):
                    pl = psmm.tile([P, 512], f32, name="ps_l", tag="mm")
                    nc.tensor.matmul(
                        pl[:],
                        sb_kT[po : po + DH, jc * P : (jc + 1) * P],
                        sb_qT[po : po + DH, ic * 512 : (ic + 1) * 512],
                        start=True,
                        stop=True,
                    )
                    e = epool.tile([P, 512], mm_dt, name="e_t", tag="e")
                    nc.scalar.activation(
                        e[:], pl[:], mybir.ActivationFunctionType.Exp
                    )
                    r = jc - 4 * ic
                    if r >= 0:
                        nc.vector.tensor_mul(e[:], e[:], sb_mask[:, r, :])
                    es.append(e)
                acc = psacc.tile([DH + 1, 512], f32, name="ps_acc", tag="acc")
                for jc in range(njc):
                    nc.tensor.matmul(
                        acc[:],
                        sb_v[:, jc, h * 65 : (h + 1) * 65],
                        es[jc][:],
                        start=(jc == 0),
                        stop=(jc == njc - 1),
                    )
                k = h * NB512 + ic
                dst = sb_attnT[po : po + DH, ic * 512 : (ic + 1) * 512]
                nc.scalar.copy(dst, acc[0:DH, :])
                rck = rcpool.tile([1, 512], f32, name="rck", tag=f"rck{k}")
                nc.vector.reciprocal(rck[:], acc[DH : DH + 1, :])
                rcs[k] = rck

        # softmax normalization broadcasts (reciprocals already done above)
        for ic in range(NB512):
            for h in (0, 1):
                po = h * DH
                k = h * NB512 + ic
                bc = psbc.tile([DH, 512], f32, name="ps_bc", tag="bc")
                nc.tensor.matmul(bc[:], sb_ones[:], rcs[k][:], start=True, stop=True)
                dst = sb_attnT[po : po + DH, ic * 512 : (ic + 1) * 512]
                nc.vector.tensor_mul(dst, dst, bc[:])

        # output projection: out[s, e] = sum_d attnT[d, s] * Wout[d, e]
        for sc in range(NB128):
            for ec in range(D // 512):
                pp = psmm.tile([P, 512], f32, name="ps_p", tag="mm")
                nc.tensor.matmul(
                    pp[:],
                    sb_attnT[:, sc * P : (sc + 1) * P],
                    sb_wout[:, ec * 512 : (ec + 1) * 512],
                    start=True,
                    stop=True,
                )
                ot = opool.tile([P, 512], F32, name="ot", tag="ot")
                if (sc * 2 + ec) % 2 == 0:
                    nc.vector.tensor_copy(ot[:], pp[:])
                else:
                    nc.scalar.copy(ot[:], pp[:])
                nc.sync.dma_start(
                    out[sc * P : (sc + 1) * P, ec * 512 : (ec + 1) * 512], ot[:]
                )


def build(mm_dt=F32):
    key = str(mm_dt)
    if key in _compiled:
        return _compiled[key]
    nc = bacc.Bacc("TRN2", target_bir_lowering=False, debug=False, num_devices=N_CORES)
    xt = nc.dram_tensor("xt", [D, S], mm_dt, kind="ExternalInput").ap()
    w = nc.dram_tensor("w", [D, 384], mm_dt, kind="ExternalInput").ap()
    wout = nc.dram_tensor("wout", [P, D], mm_dt, kind="ExternalInput").ap()
    maskt = nc.dram_tensor("maskt", [P, 4, 512], mm_dt, kind="ExternalInput").ap()
    out = nc.dram_tensor("out", [S, D], F32, kind="ExternalOutput").ap()
    with tile.TileContext(nc) as tc:
        _emit(nc, tc, mm_dt, xt, w, wout, maskt, out)
    nc.compile()
    _compiled[key] = nc
    return nc


def make_inputs(x, Wqkv, Wout):
    """Host-side shard/layout prep -> per-core input maps."""
    x = np.ascontiguousarray(np.asarray(x, np.float32))
    Wqkv = np.asarray(Wqkv, np.float32)
    Wout = np.asarray(Wout, np.float32)
    xT = np.ascontiguousarray(x.T)  # [D, S]
    j = np.arange(512, dtype=np.int64)
    m512 = (j[:, None] <= j[None, :]).astype(np.float32)  # [J, i]: J <= i
    mask = np.ascontiguousarray(
        m512.reshape(4, 128, 512).transpose(1, 0, 2)
    )  # [p, r, i] = (128r + p <= i)
    in_maps = []
    for c in range(N_CORES):
        wq = Wqkv[:, 128 * c : 128 * (c + 1)] * (1.0 / np.sqrt(DH))
        wk = Wqkv[:, D + 128 * c : D + 128 * (c + 1)]
        wv = Wqkv[:, 2 * D + 128 * c : 2 * D + 128 * (c + 1)]
        w_loc = np.ascontiguousarray(np.concatenate([wq, wk, wv], axis=1))
        wout_loc = np.ascontiguousarray(Wout[128 * c : 128 * (c + 1), :])
        in_maps.append(
            {
                "xt": xT,
                "w": w_loc.astype(np.float32),
                "wout": wout_loc,
                "maskt": mask,
            }
        )
    return in_maps


def kernel(x, Wqkv, Wout, bias, mm_dt=F32, **run_kwargs):
    nc = build(mm_dt)
    in_maps = make_inputs(x, Wqkv, Wout)
    res = run_bass_kernel_spmd(nc, in_maps, core_ids=list(range(N_CORES)), **run_kwargs)
    acc = np.zeros((S, D), np.float64)
    for c in range(N_CORES):
        acc += res.results[c]["out"].astype(np.float64)
    acc += np.asarray(bias, np.float64)[None, :]
    return acc.astype(np.float32)


# revision 20
# speedup vs baseline: 1.2414x; 1.2135x over previous
"""Causal self-attention (S=2048, D=1024, 16 heads x 64) on 8 Trainium2 cores.

Tensor-parallel sharding: 2 heads per core. Each core computes
  qkv_local = x @ Wqkv[:, local]      (local q/k/v columns, q pre-scaled 1/8)
  attn_h    = softmax(mask(q_h k_h^T)) v_h          for its 2 heads
  partial   = concat(attn) @ Wout[local_rows, :]    (128 rows of Wout)
and the host sums the 8 partials (+bias).

On-chip layout: q^T/k^T are [128, S] with head-dim on partitions (h0 rows
0-63, h1 rows 64-127) so the two heads' logit matmuls (K=64 contractions)
run concurrently in different PE row-groups. Logits are computed
transposed ([key, query]) so that exp(logits) blocks feed the
probs@v matmul directly as the moving operand; v carries an appended
ones-column so the same accumulation also produces the softmax row-sums.
No max-subtraction is needed: logits are ~N(0,1) after the 1/8 scale, so
exp() is well within fp32 range, and masked entries are zeroed after exp.
"""

import numpy as np

import concourse.bass as bass
import concourse.mybir as mybir
import concourse.tile as tile
from concourse import bacc
from concourse.bass_utils import run_bass_kernel_spmd

S = 2048
D = 1024
DH = 64
N_CORES = 8

P = 128
NB512 = S // 512  # 512-wide query chunks
NB128 = S // 128  # 128-wide chunks
KO = D // P  # contraction chunks for the projections

F32 = mybir.dt.float32

_compiled = {}


def _emit(nc, tc, mm_dt, xt, w, wout, maskt, out):
    f32 = F32
    with (
        tc.tile_pool(name="const", bufs=1) as const,
        tc.tile_pool(name="epool", bufs=18) as epool,
        tc.tile_pool(name="opool", bufs=4) as opool,
        tc.tile_pool(name="rcpool", bufs=1) as rcpool,
        tc.tile_pool(name="psmm", bufs=4, space="PSUM") as psmm,
        tc.tile_pool(name="psacc", bufs=2, space="PSUM") as psacc,
        tc.tile_pool(name="psbc", bufs=1, space="PSUM") as psbc,
    ):
        sb_xT = const.tile([P, KO, S], mm_dt, name="sb_xT")
        sb_w = const.tile([P, KO, 384], mm_dt, name="sb_w")
        sb_wout = const.tile([P, D], mm_dt, name="sb_wout")
        sb_mask = const.tile([P, 4, 512], mm_dt, name="sb_mask")
        # per-head q^T/k^T padded to K=128 with zero rows 64-127: keeps the
        # logit matmuls at full contraction width (K=64 f32r matmuls run
        # ~1.6x slower and do not register PE activity, leaving the clock
        # gate cold for the whole attention phase)
        sb_qT = [const.tile([P, S], mm_dt, name=f"sb_qT{h}") for h in (0, 1)]
        sb_kT = [const.tile([P, S], mm_dt, name=f"sb_kT{h}") for h in (0, 1)]
        sb_v = const.tile([P, NB128, 130], mm_dt, name="sb_v")
        sb_attnT = const.tile([P, S], mm_dt, name="sb_attnT")
        sb_ones = const.tile([1, DH], f32, name="sb_ones")

        # loads (per-chunk so compute can start early)
        for o in range(KO):
            nc.sync.dma_start(sb_xT[:, o, :], xt[o * P : (o + 1) * P, :])
            nc.sync.dma_start(sb_w[:, o, :], w[o * P : (o + 1) * P, :])
        nc.sync.dma_start(sb_wout[:], wout[:])
        nc.sync.dma_start(sb_mask[:], maskt[:])
        nc.gpsimd.memset(sb_ones[:], 1.0)
        nc.gpsimd.memset(sb_v[:, :, DH].bitcast(F32), 1.0)
        nc.gpsimd.memset(sb_v[:, :, 129].bitcast(F32), 1.0)
        for h in (0, 1):
            nc.gpsimd.memset(sb_qT[h][DH:P, :].bitcast(F32), 0.0)
            nc.gpsimd.memset(sb_kT[h][DH:P, :].bitcast(F32), 0.0)

        # q^T / k^T : [c, s] = sum_D W[D, c] * xT[D, s]
        for cc, dest in ((0, sb_qT), (1, sb_kT)):
            for si in range(NB512):
                ps = psmm.tile([P, 512], f32, name="ps_qk", tag="mm")
                for o in range(KO):
                    nc.tensor.matmul(
                        ps[:],
                        sb_w[:, o, cc * P : (cc + 1) * P],
                        sb_xT[:, o, si * 512 : (si + 1) * 512],
                        start=(o == 0),
                        stop=(o == KO - 1),
                    )
                sl = slice(si * 512, (si + 1) * 512)
                nc.vector.tensor_copy(dest[0][0:DH, sl], ps[0:DH, :])
                nc.vector.tensor_copy(dest[1][0:DH, sl], ps[DH:P, :])

        # v natural: [s, d] = sum_D xT[D, s] * Wv[D, d]  (both heads: 128 cols)
        for sc in range(NB128):
            psv = psmm.tile([P, 512], f32, name="ps_v", tag="mm")
            for o in range(KO):
                nc.tensor.matmul(
                    psv[:, :P],
                    sb_xT[:, o, sc * P : (sc + 1) * P],
                    sb_w[:, o, 256:384],
                    start=(o == 0),
                    stop=(o == KO - 1),
                )
            nc.vector.tensor_copy(sb_v[:, sc, 0:DH], psv[:, 0:DH])
            nc.vector.tensor_copy(sb_v[:, sc, DH + 1 : 129], psv[:, DH:P])

        # attention, causal-skipped, logits kept transposed [j, i]
        rcs = {}
        for ic in range(NB512):
            for h in (0, 1):
                po = h * DH
                njc = 4 * (ic + 1)
                es = []
                for jc in range(njc):
                    pl = psmm.tile([P, 512], f32, name="ps_l", tag="mm")
                    nc.tensor.matmul(
                        pl[:],
                        sb_kT[h][:, jc * P : (jc + 1) * P],
                        sb_qT[h][:, ic * 512 : (ic + 1) * 512],
                        start=True,
                        stop=True,
                    )
                    e = epool.tile([P, 512], mm_dt, name="e_t", tag="e")
                    nc.scalar.activation(
                        e[:], pl[:], mybir.ActivationFunctionType.Exp
                    )
                    r = jc - 4 * ic
                    if r >= 0:
                        nc.vector.tensor_mul(e[:], e[:], sb_mask[:, r, :])
                    es.append(e)
                acc = psacc.tile([DH + 1, 512], f32, name="ps_acc", tag="acc")
                for jc in range(njc):
                    nc.tensor.matmul(
                        acc[:],
                        sb_v[:, jc, h * 65 : (h + 1) * 65],
                        es[jc][:],
                        start=(jc == 0),
                        stop=(jc == njc - 1),
                    )
                k = h * NB512 + ic
                dst = sb_attnT[po : po + DH, ic * 512 : (ic + 1) * 512]
                nc.scalar.copy(dst, acc[0:DH, :])
                rck = rcpool.tile([1, 512], f32, name="rck", tag=f"rck{k}")
                nc.vector.reciprocal(rck[:], acc[DH : DH + 1, :])
                rcs[k] = rck

        # softmax normalization broadcasts (reciprocals already done above)
        for ic in range(NB512):
            for h in (0, 1):
                po = h * DH
                k = h * NB512 + ic
                bc = psbc.tile([DH, 512], f32, name="ps_bc", tag="bc")
                nc.tensor.matmul(bc[:], sb_ones[:], rcs[k][:], start=True, stop=True)
                dst = sb_attnT[po : po + DH, ic * 512 : (ic + 1) * 512]
                nc.vector.tensor_mul(dst, dst, bc[:])

        # output projection: out[s, e] = sum_d attnT[d, s] * Wout[d, e]
        for sc in range(NB128):
            for ec in range(D // 512):
                pp = psmm.tile([P, 512], f32, name="ps_p", tag="mm")
                nc.tensor.matmul(
                    pp[:],
                    sb_attnT[:, sc * P : (sc + 1) * P],
                    sb_wout[:, ec * 512 : (ec + 1) * 512],
                    start=True,
                    stop=True,
                )
                ot = opool.tile([P, 512], F32, name="ot", tag="ot")
                if (sc * 2 + ec) % 2 == 0:
                    nc.vector.tensor_copy(ot[:], pp[:])
                else:
                    nc.scalar.copy(ot[:], pp[:])
                nc.sync.dma_start(
                    out[sc * P : (sc + 1) * P, ec * 512 : (ec + 1) * 512], ot[:]
                )


def build(mm_dt=F32):
    key = str(mm_dt)
    if key in _compiled:
        return _compiled[key]
    nc = bacc.Bacc("TRN2", target_bir_lowering=False, debug=False, num_devices=N_CORES)
    xt = nc.dram_tensor("xt", [D, S], mm_dt, kind="ExternalInput").ap()
    w = nc.dram_tensor("w", [D, 384], mm_dt, kind="ExternalInput").ap()
    wout = nc.dram_tensor("wout", [P, D], mm_dt, kind="ExternalInput").ap()
    maskt = nc.dram_tensor("maskt", [P, 4, 512], mm_dt, kind="ExternalInput").ap()
    out = nc.dram_tensor("out", [S, D], F32, kind="ExternalOutput").ap()
    with tile.TileContext(nc) as tc:
        _emit(nc, tc, mm_dt, xt, w, wout, maskt, out)
    nc.compile()
    _compiled[key] = nc
    return nc


def make_inputs(x, Wqkv, Wout):
    """Host-side shard/layout prep -> per-core input maps."""
    x = np.ascontiguousarray(np.asarray(x, np.float32))
    Wqkv = np.asarray(Wqkv, np.float32)
    Wout = np.asarray(Wout, np.float32)
    xT = np.ascontiguousarray(x.T)  # [D, S]
    j = np.arange(512, dtype=np.int64)
    m512 = (j[:, None] <= j[None, :]).astype(np.float32)  # [J, i]: J <= i
    mask = np.ascontiguousarray(
        m512.reshape(4, 128, 512).transpose(1, 0, 2)
    )  # [p, r, i] = (128r + p <= i)
    in_maps = []
    for c in range(N_CORES):
        wq = Wqkv[:, 128 * c : 128 * (c + 1)] * (1.0 / np.sqrt(DH))
        wk = Wqkv[:, D + 128 * c : D + 128 * (c + 1)]
        wv = Wqkv[:, 2 * D + 128 * c : 2 * D + 128 * (c + 1)]
        w_loc = np.ascontiguousarray(np.concatenate([wq, wk, wv], axis=1))
        wout_loc = np.ascontiguousarray(Wout[128 * c : 128 * (c + 1), :])
        in_maps.append(
            {
                "xt": xT,
                "w": w_loc.astype(np.float32),
                "wout": wout_loc,
                "maskt": mask,
            }
        )
    return in_maps


def kernel(x, Wqkv, Wout, bias, mm_dt=F32, **run_kwargs):
    nc = build(mm_dt)
    in_maps = make_inputs(x, Wqkv, Wout)
    res = run_bass_kernel_spmd(nc, in_maps, core_ids=list(range(N_CORES)), **run_kwargs)
    acc = np.zeros((S, D), np.float64)
    for c in range(N_CORES):
        acc += res.results[c]["out"].astype(np.float64)
    acc += np.asarray(bias, np.float64)[None, :]
    return acc.astype(np.float32)


# revision 23
# speedup vs baseline: 1.3183x; 1.0620x over previous
"""Causal self-attention (S=2048, D=1024, 16 heads x 64) on 8 Trainium2 cores.

Tensor-parallel sharding: 2 heads per core. Each core computes
  qkv_local = x @ Wqkv[:, local]      (local q/k/v columns, q pre-scaled 1/8)
  attn_h    = softmax(mask(q_h k_h^T)) v_h          for its 2 heads
  partial   = concat(attn) @ Wout[local_rows, :]    (128 rows of Wout)
and the host sums the 8 partials (+bias).

On-chip layout: q^T/k^T are [128, S] with head-dim on partitions (h0 rows
0-63, h1 rows 64-127) so the two heads' logit matmuls (K=64 contractions)
run concurrently in different PE row-groups. Logits are computed
transposed ([key, query]) so that exp(logits) blocks feed the
probs@v matmul directly as the moving operand; v carries an appended
ones-column so the same accumulation also produces the softmax row-sums.
No max-subtraction is needed: logits are ~N(0,1) after the 1/8 scale, so
exp() is well within fp32 range, and masked entries are zeroed after exp.
"""

import numpy as np

import concourse.bass as bass
import concourse.mybir as mybir
import concourse.tile as tile
from concourse import bacc
from concourse.bass_utils import run_bass_kernel_spmd

S = 2048
D = 1024
DH = 64
N_CORES = 8

P = 128
NB512 = S // 512  # 512-wide query chunks
NB128 = S // 128  # 128-wide chunks
KO = D // P  # contraction chunks for the projections

F32 = mybir.dt.float32

_compiled = {}


def _emit(nc, tc, mm_dt, xt, w, wout, maskt, out):
    f32 = F32
    with (
        tc.tile_pool(name="const", bufs=1) as const,
        tc.tile_pool(name="epool", bufs=18) as epool,
        tc.tile_pool(name="opool", bufs=4) as opool,
        tc.tile_pool(name="rcpool", bufs=1) as rcpool,
        tc.tile_pool(name="psmm", bufs=4, space="PSUM") as psmm,
        tc.tile_pool(name="psacc", bufs=2, space="PSUM") as psacc,
    ):
        sb_xT = const.tile([P, KO, S], mm_dt, name="sb_xT")
        sb_w = const.tile([P, KO, 384], mm_dt, name="sb_w")
        sb_wout = const.tile([P, D], mm_dt, name="sb_wout")
        sb_mask = const.tile([P, 4, 512], mm_dt, name="sb_mask")
        # per-head q^T/k^T padded to K=128 with zero rows 64-127: keeps the
        # logit matmuls at full contraction width (K=64 f32r matmuls run
        # ~1.6x slower and do not register PE activity, leaving the clock
        # gate cold for the whole attention phase)
        sb_qT = [const.tile([P, S], mm_dt, name=f"sb_qT{h}") for h in (0, 1)]
        sb_kT = [const.tile([P, S], mm_dt, name=f"sb_kT{h}") for h in (0, 1)]
        sb_v = const.tile([P, NB128, 130], mm_dt, name="sb_v")
        sb_attnT = const.tile([P, S], mm_dt, name="sb_attnT")

        # loads (per-chunk so compute can start early)
        for o in range(KO):
            nc.sync.dma_start(sb_w[:, o, :], w[o * P : (o + 1) * P, :])
            for si in range(NB512):
                sl = slice(si * 512, (si + 1) * 512)
                nc.sync.dma_start(sb_xT[:, o, sl], xt[o * P : (o + 1) * P, sl])
        nc.sync.dma_start(sb_wout[:], wout[:])
        nc.sync.dma_start(sb_mask[:], maskt[:])
        nc.gpsimd.memset(sb_v[:, :, DH].bitcast(F32), 1.0)
        nc.gpsimd.memset(sb_v[:, :, 129].bitcast(F32), 1.0)
        for h in (0, 1):
            nc.gpsimd.memset(sb_qT[h][DH:P, :].bitcast(F32), 0.0)
            nc.gpsimd.memset(sb_kT[h][DH:P, :].bitcast(F32), 0.0)

        # q^T / k^T : [c, s] = sum_D W[D, c] * xT[D, s]
        for cc, dest in ((0, sb_qT), (1, sb_kT)):
            for si in range(NB512):
                ps = psmm.tile([P, 512], f32, name="ps_qk", tag="mm")
                for o in range(KO):
                    nc.tensor.matmul(
                        ps[:],
                        sb_w[:, o, cc * P : (cc + 1) * P],
                        sb_xT[:, o, si * 512 : (si + 1) * 512],
                        start=(o == 0),
                        stop=(o == KO - 1),
                    )
                sl = slice(si * 512, (si + 1) * 512)
                nc.vector.tensor_copy(dest[0][0:DH, sl], ps[0:DH, :])
                nc.vector.tensor_copy(dest[1][0:DH, sl], ps[DH:P, :])

        # v natural: [s, d] = sum_D xT[D, s] * Wv[D, d]  (both heads: 128 cols)
        for sc in range(NB128):
            psv = psmm.tile([P, 512], f32, name="ps_v", tag="mm")
            for o in range(KO):
                nc.tensor.matmul(
                    psv[:, :P],
                    sb_xT[:, o, sc * P : (sc + 1) * P],
                    sb_w[:, o, 256:384],
                    start=(o == 0),
                    stop=(o == KO - 1),
                )
            nc.vector.tensor_copy(sb_v[:, sc, 0:DH], psv[:, 0:DH])
            nc.vector.tensor_copy(sb_v[:, sc, DH + 1 : 129], psv[:, DH:P])

        # attention, causal-skipped, logits kept transposed [j, i]
        for ic in range(NB512):
            for h in (0, 1):
                po = h * DH
                njc = 4 * (ic + 1)
                es = []
                for jc in range(njc):
                    pl = psmm.tile([P, 512], f32, name="ps_l", tag="mm")
                    nc.tensor.matmul(
                        pl[:],
                        sb_kT[h][:, jc * P : (jc + 1) * P],
                        sb_qT[h][:, ic * 512 : (ic + 1) * 512],
                        start=True,
                        stop=True,
                    )
                    e = epool.tile([P, 512], mm_dt, name="e_t", tag="e")
                    nc.scalar.activation(
                        e[:], pl[:], mybir.ActivationFunctionType.Exp
                    )
                    r = jc - 4 * ic
                    if r >= 0:
                        nc.vector.tensor_mul(e[:], e[:], sb_mask[:, r, :])
                    es.append(e)
                acc = psacc.tile([DH + 1, 512], f32, name="ps_acc", tag="acc")
                for jc in range(njc):
                    nc.tensor.matmul(
                        acc[:],
                        sb_v[:, jc, h * 65 : (h + 1) * 65],
                        es[jc][:],
                        start=(jc == 0),
                        stop=(jc == njc - 1),
                    )
                # normalize: reciprocal of the rowsum row, broadcast across
                # partitions on the (otherwise idle) GpSimd engine, then one
                # PSUM-reading multiply straight into attnT
                k = h * NB512 + ic
                rck = rcpool.tile([1, 512], f32, name="rck", tag=f"rck{k}")
                nc.vector.reciprocal(rck[:], acc[DH : DH + 1, :])
                bck = rcpool.tile([DH, 512], f32, name="bck", tag=f"bck{k % 4}")
                nc.gpsimd.partition_broadcast(bck[:], rck[:])
                dst = sb_attnT[po : po + DH, ic * 512 : (ic + 1) * 512]
                nc.vector.tensor_mul(dst, acc[0:DH, :], bck[:])

        # output projection: out[s, e] = sum_d attnT[d, s] * Wout[d, e]
        for sc in range(NB128):
            for ec in range(D // 512):
                pp = psmm.tile([P, 512], f32, name="ps_p", tag="mm")
                nc.tensor.matmul(
                    pp[:],
                    sb_attnT[:, sc * P : (sc + 1) * P],
                    sb_wout[:, ec * 512 : (ec + 1) * 512],
                    start=True,
                    stop=True,
                )
                ot = opool.tile([P, 512], F32, name="ot", tag="ot")
                if (sc * 2 + ec) % 2 == 0:
                    nc.vector.tensor_copy(ot[:], pp[:])
                else:
                    nc.scalar.copy(ot[:], pp[:])
                nc.sync.dma_start(
                    out[sc * P : (sc + 1) * P, ec * 512 : (ec + 1) * 512], ot[:]
                )


def build(mm_dt=F32):
    key = str(mm_dt)
    if key in _compiled:
        return _compiled[key]
    nc = bacc.Bacc("TRN2", target_bir_lowering=False, debug=False, num_devices=N_CORES)
    xt = nc.dram_tensor("xt", [D, S], mm_dt, kind="ExternalInput").ap()
    w = nc.dram_tensor("w", [D, 384], mm_dt, kind="ExternalInput").ap()
    wout = nc.dram_tensor("wout", [P, D], mm_dt, kind="ExternalInput").ap()
    maskt = nc.dram_tensor("maskt", [P, 4, 512], mm_dt, kind="ExternalInput").ap()
    out = nc.dram_tensor("out", [S, D], F32, kind="ExternalOutput").ap()
    with tile.TileContext(nc) as tc:
        _emit(nc, tc, mm_dt, xt, w, wout, maskt, out)
    nc.compile()
    _compiled[key] = nc
    return nc


def make_inputs(x, Wqkv, Wout):
    """Host-side shard/layout prep -> per-core input maps."""
    x = np.ascontiguousarray(np.asarray(x, np.float32))
    Wqkv = np.asarray(Wqkv, np.float32)
    Wout = np.asarray(Wout, np.float32)
    xT = np.ascontiguousarray(x.T)  # [D, S]
    j = np.arange(512, dtype=np.int64)
    m512 = (j[:, None] <= j[None, :]).astype(np.float32)  # [J, i]: J <= i
    mask = np.ascontiguousarray(
        m512.reshape(4, 128, 512).transpose(1, 0, 2)
    )  # [p, r, i] = (128r + p <= i)
    in_maps = []
    for c in range(N_CORES):
        wq = Wqkv[:, 128 * c : 128 * (c + 1)] * (1.0 / np.sqrt(DH))
        wk = Wqkv[:, D + 128 * c : D + 128 * (c + 1)]
        wv = Wqkv[:, 2 * D + 128 * c : 2 * D + 128 * (c + 1)]
        w_loc = np.ascontiguousarray(np.concatenate([wq, wk, wv], axis=1))
        wout_loc = np.ascontiguousarray(Wout[128 * c : 128 * (c + 1), :])
        in_maps.append(
            {
                "xt": xT,
                "w": w_loc.astype(np.float32),
                "wout": wout_loc,
                "maskt": mask,
            }
        )
    return in_maps


def kernel(x, Wqkv, Wout, bias, mm_dt=F32, **run_kwargs):
    nc = build(mm_dt)
    in_maps = make_inputs(x, Wqkv, Wout)
    res = run_bass_kernel_spmd(nc, in_maps, core_ids=list(range(N_CORES)), **run_kwargs)
    acc = np.zeros((S, D), np.float64)
    for c in range(N_CORES):
        acc += res.results[c]["out"].astype(np.float64)
    acc += np.asarray(bias, np.float64)[None, :]
    return acc.astype(np.float32)
